# revision 17
# baseline (speedup 1.0000x reference)
"""Trainium2 Bass kernel for nn_AttentionSheafLearner.

Computation:  maps = x[row] @ W[:, :C].T + x[col] @ W[:, C:].T    [E, 25]
              out  = eye(5) - softmax(maps.reshape(E, 5, 5), axis=-1)

Strategy (8 NeuronCores, SPMD):
  - Precompute z[n] = [x[n] @ Wr.T | x[n] @ Wc.T | pad]  (64 f32 = 256B rows)
    on device with bf16 PE matmuls; per node-half tables in DRAM.
  - Edges sharded by VALUE class: nodes split in two halves (A = <25088),
    edge class = (row_half, col_half); each class handled by 2 cores so
    per-core gather indices fit int16 (SWDGE dma_gather idx dtype).
  - Row-side gather dedup: edges are grouped by row node into multiplicity
    classes (class k = groups of k edges sharing one row). The row z is
    gathered ONCE per group and expanded on-chip with a stride-0 broadcast
    AP in the DVE add; only the col side is gathered per edge. This cuts
    SWDGE descriptor-generation work (the GPSIMD bottleneck) ~1.7x.
  - Per chunk: dma_gather row groups + per-edge cols, DVE add -> maps,
    ScalarE exp, DVE strided reduce / reciprocal / broadcast-mul,
    out = eye - sm, store.
  - Host re-permutes the output rows back to original edge order.

The class layout (quotas per multiplicity class) is computed from the actual
edge_index at kernel() time; the program is built and compiled per call.
"""

import os

import numpy as np

# problem sizes (hardcoded per contract)
N = 50000
C = 128
D = 5
DD = D * D          # 25
E = 1_600_000
NCORES = 8
P = 128

HALF = 25088        # nodes per half (padded; 2*HALF >= N)
ZW = 64             # z row width in f32 (256B, dma_gather elem size)
NCH_H = HALF // P   # 196 node chunks per half
KMAX = 10           # max row-multiplicity class; larger rows are decomposed
COL_TARGET = 64     # target col-slot columns per chunk (col idxs ~ 128*64)

_XBLK = 28          # node chunks per xT DMA block
_ZGRP = 14          # node chunks per z store group


def _su_for(kk):
    return max(1, round(COL_TARGET / kk))


class _Layout:
    """Per-run static layout shared by host prep and program build.

    quotas[k] = padded group count (multiple of 128) for class k, the max
    over cores. chunks: list of (kk, su, u0, n_u, r_off, c_off, o_off) with
    u0/n_u in group-columns (128 groups per column), r_off/c_off in idx-tile
    int16 columns (16-wrapped), o_off in output slot-columns.
    """

    def __init__(self, quotas):
        self.quotas = dict(quotas)           # kk -> padded group count
        self.chunks = []
        self.class_cols = {}                 # kk -> starting out slot-column
        r_off = c_off = o_off = 0
        for kk in sorted(self.quotas, reverse=True):
            q = self.quotas[kk]
            assert q % P == 0
            su_full = _su_for(kk)
            self.class_cols[kk] = o_off
            ncols = q // P
            u0 = 0
            while u0 < ncols:
                su = min(su_full, ncols - u0)
                self.chunks.append((kk, su, u0, r_off, c_off, o_off))
                r_off += (P * su) // 16
                c_off += (P * su * kk) // 16
                o_off += su * kk
                u0 += su
        self.r_cols = r_off
        self.c_cols = c_off
        self.t_cols = o_off                  # total output slot-columns


def _build_layout(row_locals):
    """row_locals: list per core of int arrays (local row ids). Returns
    (_Layout, groups_per_core) where groups_per_core[core][kk] is a list of
    (row_id, edge_positions array of length kk)."""
    per_core = []
    quotas = {}
    for rl in row_locals:
        order = np.argsort(rl, kind="stable")
        sr = rl[order]
        # boundaries of equal runs
        starts = np.flatnonzero(np.r_[True, sr[1:] != sr[:-1]])
        ends = np.r_[starts[1:], len(sr)]
        groups = {}
        for s, e in zip(starts, ends):
            r = int(sr[s])
            pos = order[s:e]
            o = 0
            k = e - s
            while k > 0:
                kk = min(k, KMAX)
                groups.setdefault(kk, []).append((r, pos[o:o + kk]))
                o += kk
                k -= kk
        per_core.append(groups)
        for kk, g in groups.items():
            quotas[kk] = max(quotas.get(kk, 0), len(g))
    for kk in quotas:
        quotas[kk] = -(-quotas[kk] // P) * P
    return _Layout(quotas), per_core


def _build_nc(layout):
    from contextlib import ExitStack

    import concourse.bacc as bacc
    import concourse.mybir as mybir
    import concourse.tile as tile

    f32 = mybir.dt.float32
    bf16 = mybir.dt.bfloat16
    i16 = mybir.dt.int16

    nc = bacc.Bacc(
        "TRN2",
        target_bir_lowering=False,
        debug=False,
        enable_asserts=False,
        num_devices=NCORES,
        num_swdge_queues=4,
    )

    xt_r_d = nc.dram_tensor("xt_r", [P, HALF], bf16, kind="ExternalInput")
    xt_c_d = nc.dram_tensor("xt_c", [P, HALF], bf16, kind="ExternalInput")
    w_d = nc.dram_tensor("w", [P, 2 * DD], bf16, kind="ExternalInput")
    ridx_d = nc.dram_tensor("ridx", [P, layout.r_cols], i16, kind="ExternalInput")
    cidx_d = nc.dram_tensor("cidx", [P, layout.c_cols], i16, kind="ExternalInput")
    eye_d = nc.dram_tensor("eye", [P, DD], f32, kind="ExternalInput")
    z_r_d = nc.dram_tensor("z_r", [HALF, ZW], f32)
    z_c_d = nc.dram_tensor("z_c", [HALF, ZW], f32)
    out_d = nc.dram_tensor("out", [P * layout.t_cols, DD], f32, kind="ExternalOutput")

    oview = out_d.ap().rearrange("(p t) d -> p (t d)", p=P)  # [128, t_cols*25]

    with tile.TileContext(nc) as tc, ExitStack() as ctx:
        const_pool = ctx.enter_context(tc.tile_pool(name="const", bufs=1))
        w_tile = const_pool.tile([P, 2 * DD], bf16)
        nc.sync.dma_start(w_tile[:], w_d.ap())
        eye_tile = const_pool.tile([P, DD], f32)
        nc.sync.dma_start(eye_tile[:], eye_d.ap())

        # ---- stage A: z tables (bf16 matmuls, f32 tables) ----
        with ExitStack() as actx:
            xt_pool = actx.enter_context(tc.tile_pool(name="xt", bufs=2))
            z_pool = actx.enter_context(tc.tile_pool(name="zsb", bufs=3))
            ps_pool = actx.enter_context(
                tc.tile_pool(name="ps", bufs=8, space="PSUM")
            )
            for xt_d, z_d in ((xt_r_d, z_r_d), (xt_c_d, z_c_d)):
                zview = z_d.ap().rearrange("(i p) d -> i p d", p=P)
                for blk in range(NCH_H // _XBLK):  # 7
                    xt_tile = xt_pool.tile([P, _XBLK * P], bf16)
                    nc.sync.dma_start(
                        xt_tile[:],
                        xt_d.ap()[:, blk * _XBLK * P:(blk + 1) * _XBLK * P],
                    )
                    for grp in range(_XBLK // _ZGRP):  # 2
                        z_sb = z_pool.tile([P, _ZGRP * ZW], f32)
                        for j in range(_ZGRP):
                            jj = grp * _ZGRP + j
                            ps = ps_pool.tile([P, 2 * DD], f32, space="PSUM")
                            nc.tensor.matmul(
                                ps[:],
                                xt_tile[:, jj * P:(jj + 1) * P],
                                w_tile[:],
                                start=True,
                                stop=True,
                            )
                            nc.vector.tensor_copy(
                                z_sb[:, j * ZW:j * ZW + 2 * DD], ps[:]
                            )
                        i0 = blk * _XBLK + grp * _ZGRP
                        nc.sync.dma_start(
                            zview[i0:i0 + _ZGRP, :, :].rearrange("i p d -> p i d"),
                            z_sb[:].rearrange("p (i d) -> p i d", i=_ZGRP),
                        )

        # ---- stage B: gather + softmax per class chunk ----
        gr_pool = ctx.enter_context(tc.tile_pool(name="gr", bufs=10))
        gc_pool = ctx.enter_context(tc.tile_pool(name="gc", bufs=4))
        i_pool = ctx.enter_context(tc.tile_pool(name="ix", bufs=12))
        m_pool = ctx.enter_context(tc.tile_pool(name="m", bufs=2))
        e_pool = ctx.enter_context(tc.tile_pool(name="e", bufs=2))
        s_pool = ctx.enter_context(tc.tile_pool(name="s", bufs=2))
        o_pool = ctx.enter_context(tc.tile_pool(name="o", bufs=2))
        import itertools as _it
        _qc = _it.count()
        qctr = type("Q", (), {"pop": staticmethod(lambda: next(_qc))})()
        NFRONT = 6

        def emit_row_gather(ci):
            kk, su, u0, r_off, c_off, o_off = layout.chunks[ci]
            n_r = P * su
            ri = i_pool.tile([P, n_r // 16], i16, tag="ri")
            nc.sync.dma_start(
                ri[:], ridx_d.ap()[:, r_off:r_off + n_r // 16]
            )
            g_r = gr_pool.tile([P, su * ZW], f32, tag="gr")
            nc.gpsimd.dma_gather(
                out_ap=g_r[:].rearrange("p (u d) -> p u d", d=ZW),
                in_ap=z_r_d.ap(),
                idxs_ap=ri[:],
                num_idxs=n_r,
                num_idxs_reg=n_r,
                elem_size=ZW,
                single_packet=False,
                queue_num=qctr.pop() % 4,
            )
            return g_r

        front = [emit_row_gather(ci) for ci in range(min(NFRONT, len(layout.chunks)))]
        for ci, (kk, su, u0, r_off, c_off, o_off) in enumerate(layout.chunks):
            n_r = P * su
            n_c = P * su * kk
            X = su * kk                      # out slot-columns this chunk
            g_r = front[ci] if ci < NFRONT else emit_row_gather(ci)
            ci_t = i_pool.tile([P, n_c // 16], i16, tag="ci")
            nc.sync.dma_start(
                ci_t[:], cidx_d.ap()[:, c_off:c_off + n_c // 16]
            )
            g_c = gc_pool.tile([P, X * ZW], f32, tag="gc")
            gcv = g_c[:].rearrange("p (u d) -> p u d", d=ZW)
            nsplit = 3 if X >= 6 else (2 if X >= 2 else 1)
            bounds = [(X * j // nsplit) * P for j in range(nsplit)] + [n_c]
            for piece in range(nsplit):
                lo, hi = bounds[piece], bounds[piece + 1]
                if hi <= lo:
                    continue
                nc.gpsimd.dma_gather(
                    out_ap=gcv[:, lo // P:hi // P],
                    in_ap=z_c_d.ap(),
                    idxs_ap=ci_t[:, lo // 16:hi // 16],
                    num_idxs=hi - lo,
                    num_idxs_reg=hi - lo,
                    elem_size=ZW,
                    single_packet=False,
                    queue_num=qctr.pop() % 4,
                )
            m = m_pool.tile([P, X * DD], f32)
            nc.vector.tensor_tensor(
                out=m[:].rearrange("p (u m d) -> p u m d", u=su, m=kk),
                in0=g_r[:]
                .rearrange("p (u d) -> p u d", d=ZW)[:, :, 0:DD]
                .unsqueeze(2)
                .to_broadcast([P, su, kk, DD]),
                in1=g_c[:].rearrange("p (u m d) -> p u m d", u=su, d=ZW)[
                    :, :, :, DD:2 * DD
                ],
                op=mybir.AluOpType.add,
            )
            et = e_pool.tile([P, X * DD], f32)
            nc.scalar.activation(et[:], m[:], mybir.ActivationFunctionType.Exp)
            e3 = et[:].rearrange("p (t d) -> p t d", d=D)  # [128, X*5, 5]
            s = s_pool.tile([P, X * D], f32, tag="s")
            nc.vector.reduce_sum(s[:], e3, axis=mybir.AxisListType.X)
            r = s_pool.tile([P, X * D], f32, tag="r")
            nc.vector.reciprocal(r[:], s[:])
            o = o_pool.tile([P, X * DD], f32)
            nc.vector.tensor_tensor(
                out=o[:].rearrange("p (t d) -> p t d", d=D),
                in0=e3,
                in1=r[:].unsqueeze(2).to_broadcast([P, X * D, D]),
                op=mybir.AluOpType.mult,
            )
            nc.vector.tensor_tensor(
                out=o[:].rearrange("p (t d) -> p t d", d=DD),
                in0=eye_tile[:].unsqueeze(1).to_broadcast([P, X, DD]),
                in1=o[:].rearrange("p (t d) -> p t d", d=DD),
                op=mybir.AluOpType.subtract,
            )
            nc.sync.dma_start(
                oview[:, o_off * DD:(o_off + X) * DD], o[:]
            )

    nc.compile()
    return nc


def _wrap_idx(a):
    """Gather-order idx list [n] -> [128, n//16] int16 tile (16-wrapped,
    replicated to 128 partitions)."""
    n = len(a)
    assert n % 16 == 0
    t = a.reshape(n // 16, 16).T.astype(np.int16)   # [16, n//16]
    return np.ascontiguousarray(np.tile(t, (8, 1)))


def _host_prep(x, W, edge_index):
    """Shard edges by (row_half, col_half) class across cores; group by row
    node into multiplicity classes; build per-core inputs. Returns
    (layout, in_maps, slot_maps) where slot_maps[c] = (edge_ids, out_rows)."""
    x = np.asarray(x, dtype=np.float32)
    W = np.asarray(W, dtype=np.float32)
    ei = np.asarray(edge_index)
    row = ei[0].astype(np.int64)
    col = ei[1].astype(np.int64)

    import ml_dtypes

    bf16 = ml_dtypes.bfloat16
    xt = np.zeros((P, 2 * HALF), dtype=np.float32)
    xt[:, :N] = x.T
    xt_half = [
        np.ascontiguousarray(xt[:, :HALF].astype(bf16)),
        np.ascontiguousarray(xt[:, HALF:].astype(bf16)),
    ]

    w = np.zeros((P, 2 * DD), dtype=np.float32)
    w[:, :DD] = W[:, :C].T
    w[:, DD:2 * DD] = W[:, C:].T
    w = w.astype(bf16)
    eye = np.ascontiguousarray(
        np.broadcast_to(np.eye(D, dtype=np.float32).reshape(1, DD), (P, DD))
    )

    cls = (row >= HALF).astype(np.int64) * 2 + (col >= HALF)
    order = np.argsort(cls, kind="stable")
    counts = np.bincount(cls, minlength=4)
    starts = np.concatenate([[0], np.cumsum(counts)])

    subs = []
    row_locals = []
    for core in range(NCORES):
        k = core // 2
        sub = order[starts[k]:starts[k + 1]][core % 2::2]
        subs.append(sub)
        row_locals.append((row[sub] - (k >> 1) * HALF).astype(np.int64))

    layout, per_core_groups = _build_layout(row_locals)

    in_maps = []
    slot_maps = []
    for core in range(NCORES):
        k = core // 2
        half_r, half_c = k >> 1, k & 1
        sub = subs[core]
        col_local = (col[sub] - half_c * HALF).astype(np.int64)
        groups = per_core_groups[core]

        # per class: padded group arrays (row id + kk edge positions);
        # pads use row id -1 (trailing in each list -> SWDGE skips them)
        cls_rows = {}
        cls_edges = {}
        for kk, q in layout.quotas.items():
            g = groups.get(kk, [])
            rows_a = np.zeros(q, dtype=np.int64)
            edges_a = np.full((q, kk), -1, dtype=np.int64)
            for i, (r, pos) in enumerate(g):
                rows_a[i] = r
                edges_a[i] = pos
            cls_rows[kk] = rows_a
            cls_edges[kk] = edges_a

        # build per chunk
        r_flat = []
        c_flat = []
        eids = []
        orows = []
        t_cols = layout.t_cols
        for (kk, su, u0, r_off, c_off, o_off) in layout.chunks:
            g0 = u0 * P
            rows_a = cls_rows[kk][g0:g0 + su * P]          # [su*128]
            edges_a = cls_edges[kk][g0:g0 + su * P]        # [su*128, kk]
            r_flat.append(rows_a.astype(np.int16))          # pos = u*128+p
            # col idx order: u-major, pos = (u*kk + m)*128 + p
            e_um = edges_a.reshape(su, P, kk).transpose(0, 2, 1)  # [su, kk, P]
            cvals = np.where(e_um >= 0, col_local[np.maximum(e_um, 0)], 0)
            cvals = cvals.reshape(-1)
            c_flat.append(cvals.astype(np.int16))
            # out slot rows: edge (u, m, p) -> out row p*t_cols + (o_off + u*kk + m)
            uu, mm, pp = np.meshgrid(
                np.arange(su), np.arange(kk), np.arange(P), indexing="ij"
            )
            valid = e_um >= 0
            rws = pp * t_cols + (o_off + uu * kk + mm)
            eids.append(sub[e_um[valid]])
            orows.append(rws[valid])
        ridx = _wrap_idx(np.concatenate(r_flat))
        cidx = _wrap_idx(np.concatenate(c_flat))
        in_maps.append(
            {
                "xt_r": xt_half[half_r],
                "xt_c": xt_half[half_c],
                "w": w,
                "eye": eye,
                "ridx": ridx,
                "cidx": cidx,
            }
        )
        slot_maps.append(
            (np.concatenate(eids), np.concatenate(orows))
        )
    return layout, in_maps, slot_maps


LAST_EXEC_NS = None


def kernel(x, W, edge_index):
    global LAST_EXEC_NS
    from concourse.bass_utils import run_bass_kernel_spmd

    layout, in_maps, slot_maps = _host_prep(x, W, edge_index)
    nc = _build_nc(layout)
    trace = os.environ.get("KERNEL_TRACE", "0") == "1"
    br = run_bass_kernel_spmd(
        nc,
        in_maps,
        core_ids=list(range(NCORES)),
        trace=trace,
    )
    LAST_EXEC_NS = br.exec_time_ns

    out = np.empty((E, DD), dtype=np.float32)
    for core in range(NCORES):
        res = br.results[core]["out"]        # [P*t_cols, 25]
        eids, orows = slot_maps[core]
        out[eids] = res[orows]
    return out.reshape(E, D, D).astype(np.float32)


# revision 18
# speedup vs baseline: 1.1003x; 1.1003x over previous
"""Trainium2 Bass kernel for nn_AttentionSheafLearner.

Computation:  maps = x[row] @ W[:, :C].T + x[col] @ W[:, C:].T    [E, 25]
              out  = eye(5) - softmax(maps.reshape(E, 5, 5), axis=-1)

Strategy (8 NeuronCores, SPMD):
  - Precompute z[n] = [x[n] @ Wr.T | x[n] @ Wc.T | pad]  (64 f32 = 256B rows)
    on device with bf16 PE matmuls; per node-half tables in DRAM.
  - Edges sharded by VALUE class: nodes split in two halves (A = <25088),
    edge class = (row_half, col_half); each class handled by 2 cores so
    per-core gather indices fit int16 (SWDGE dma_gather idx dtype).
  - Row-side gather dedup: edges are grouped by row node into multiplicity
    classes (class k = groups of k edges sharing one row). The row z is
    gathered ONCE per group and expanded on-chip with a stride-0 broadcast
    AP in the DVE add; only the col side is gathered per edge. This cuts
    SWDGE descriptor-generation work (the GPSIMD bottleneck) ~1.7x.
  - Per chunk: dma_gather row groups + per-edge cols, DVE add -> maps,
    ScalarE exp, DVE strided reduce / reciprocal / broadcast-mul,
    out = eye - sm, store.
  - Host re-permutes the output rows back to original edge order.

The class layout (quotas per multiplicity class) is computed from the actual
edge_index at kernel() time; the program is built and compiled per call.
"""

import os

import numpy as np

# problem sizes (hardcoded per contract)
N = 50000
C = 128
D = 5
DD = D * D          # 25
E = 1_600_000
NCORES = 8
P = 128

HALF = 25088        # nodes per half (padded; 2*HALF >= N)
ZW = 64             # z row width in f32 (256B, dma_gather elem size)
NCH_H = HALF // P   # 196 node chunks per half
KMAX = 10           # max row-multiplicity class; larger rows are decomposed
COL_TARGET = 64     # target col-slot columns per chunk (col idxs ~ 128*64)

_XBLK = 14          # node chunks per xT DMA block
_ZGRP = 7           # node chunks per z store group


def _su_for(kk):
    return max(1, round(COL_TARGET / kk))


class _Layout:
    """Per-run static layout shared by host prep and program build.

    quotas[k] = padded group count (multiple of 128) for class k, the max
    over cores. chunks: list of (kk, su, u0, n_u, r_off, c_off, o_off) with
    u0/n_u in group-columns (128 groups per column), r_off/c_off in idx-tile
    int16 columns (16-wrapped), o_off in output slot-columns.
    """

    def __init__(self, quotas):
        self.quotas = dict(quotas)           # kk -> padded group count
        self.chunks = []
        self.class_cols = {}                 # kk -> starting out slot-column
        r_off = c_off = o_off = 0
        for kk in sorted(self.quotas, reverse=True):
            q = self.quotas[kk]
            assert q % P == 0
            su_full = _su_for(kk)
            self.class_cols[kk] = o_off
            ncols = q // P
            u0 = 0
            while u0 < ncols:
                su = min(su_full, ncols - u0)
                self.chunks.append((kk, su, u0, r_off, c_off, o_off))
                r_off += (P * su) // 16
                c_off += (P * su * kk) // 16
                o_off += su * kk
                u0 += su
        self.r_cols = r_off
        self.c_cols = c_off
        self.t_cols = o_off                  # total output slot-columns


def _build_layout(row_locals):
    """row_locals: list per core of int arrays (local row ids). Returns
    (_Layout, groups_per_core) where groups_per_core[core][kk] is a list of
    (row_id, edge_positions array of length kk)."""
    per_core = []
    quotas = {}
    for rl in row_locals:
        order = np.argsort(rl, kind="stable")
        sr = rl[order]
        # boundaries of equal runs
        starts = np.flatnonzero(np.r_[True, sr[1:] != sr[:-1]])
        ends = np.r_[starts[1:], len(sr)]
        groups = {}
        for s, e in zip(starts, ends):
            r = int(sr[s])
            pos = order[s:e]
            o = 0
            k = e - s
            while k > 0:
                kk = min(k, KMAX)
                groups.setdefault(kk, []).append((r, pos[o:o + kk]))
                o += kk
                k -= kk
        per_core.append(groups)
        for kk, g in groups.items():
            quotas[kk] = max(quotas.get(kk, 0), len(g))
    for kk in quotas:
        quotas[kk] = -(-quotas[kk] // P) * P
    return _Layout(quotas), per_core


def _build_nc(layout):
    from contextlib import ExitStack

    import concourse.bacc as bacc
    import concourse.mybir as mybir
    import concourse.tile as tile

    f32 = mybir.dt.float32
    bf16 = mybir.dt.bfloat16
    i16 = mybir.dt.int16

    nc = bacc.Bacc(
        "TRN2",
        target_bir_lowering=False,
        debug=False,
        enable_asserts=False,
        num_devices=NCORES,
        num_swdge_queues=4,
    )

    xt_r_d = nc.dram_tensor("xt_r", [P, HALF], bf16, kind="ExternalInput")
    xt_c_d = nc.dram_tensor("xt_c", [P, HALF], bf16, kind="ExternalInput")
    w_d = nc.dram_tensor("w", [P, 2 * DD], bf16, kind="ExternalInput")
    ridx_d = nc.dram_tensor("ridx", [P, layout.r_cols], i16, kind="ExternalInput")
    cidx_d = nc.dram_tensor("cidx", [P, layout.c_cols], i16, kind="ExternalInput")
    eye_d = nc.dram_tensor("eye", [P, DD], f32, kind="ExternalInput")
    z_r_d = nc.dram_tensor("z_r", [HALF, ZW], f32)
    z_c_d = nc.dram_tensor("z_c", [HALF, ZW], f32)
    out_d = nc.dram_tensor("out", [P * layout.t_cols, DD], f32, kind="ExternalOutput")

    oview = out_d.ap().rearrange("(p t) d -> p (t d)", p=P)  # [128, t_cols*25]

    with tile.TileContext(nc) as tc, ExitStack() as ctx:
        const_pool = ctx.enter_context(tc.tile_pool(name="const", bufs=1))
        w_tile = const_pool.tile([P, 2 * DD], bf16)
        nc.sync.dma_start(w_tile[:], w_d.ap())
        eye_tile = const_pool.tile([P, DD], f32)
        nc.sync.dma_start(eye_tile[:], eye_d.ap())

        # ---- stage A: z tables (bf16 matmuls, f32 tables) ----
        with ExitStack() as actx:
            xt_pool = actx.enter_context(tc.tile_pool(name="xt", bufs=2))
            z_pool = actx.enter_context(tc.tile_pool(name="zsb", bufs=3))
            ps_pool = actx.enter_context(
                tc.tile_pool(name="ps", bufs=4, space="PSUM")
            )
            for xt_d, z_d in ((xt_r_d, z_r_d), (xt_c_d, z_c_d)):
                zview = z_d.ap().rearrange("(i p) d -> i p d", p=P)
                for blk in range(NCH_H // _XBLK):  # 7
                    xt_tile = xt_pool.tile([P, _XBLK * P], bf16)
                    nc.sync.dma_start(
                        xt_tile[:],
                        xt_d.ap()[:, blk * _XBLK * P:(blk + 1) * _XBLK * P],
                    )
                    for grp in range(_XBLK // _ZGRP):  # 2
                        z_sb = z_pool.tile([P, _ZGRP * ZW], f32)
                        ps = ps_pool.tile([P, _ZGRP * 2 * DD], f32, space="PSUM")
                        for j in range(_ZGRP):
                            jj = grp * _ZGRP + j
                            nc.tensor.matmul(
                                ps[:, j * 2 * DD:(j + 1) * 2 * DD],
                                xt_tile[:, jj * P:(jj + 1) * P],
                                w_tile[:],
                                start=True,
                                stop=True,
                            )
                        nc.vector.tensor_copy(
                            z_sb[:]
                            .rearrange("p (i d) -> p i d", i=_ZGRP)[:, :, 0:2 * DD],
                            ps[:].rearrange("p (i d) -> p i d", i=_ZGRP),
                        )
                        i0 = blk * _XBLK + grp * _ZGRP
                        nc.sync.dma_start(
                            zview[i0:i0 + _ZGRP, :, :].rearrange("i p d -> p i d"),
                            z_sb[:].rearrange("p (i d) -> p i d", i=_ZGRP),
                        )

        # ---- stage B: gather + softmax per class chunk ----
        gr_pool = ctx.enter_context(tc.tile_pool(name="gr", bufs=10))
        gc_pool = ctx.enter_context(tc.tile_pool(name="gc", bufs=4))
        i_pool = ctx.enter_context(tc.tile_pool(name="ix", bufs=12))
        m_pool = ctx.enter_context(tc.tile_pool(name="m", bufs=2))
        e_pool = ctx.enter_context(tc.tile_pool(name="e", bufs=2))
        s_pool = ctx.enter_context(tc.tile_pool(name="s", bufs=2))
        o_pool = ctx.enter_context(tc.tile_pool(name="o", bufs=2))
        import itertools as _it
        _qc = _it.count()
        qctr = type("Q", (), {"pop": staticmethod(lambda: next(_qc))})()
        NFRONT = 6

        def emit_row_gather(ci):
            kk, su, u0, r_off, c_off, o_off = layout.chunks[ci]
            n_r = P * su
            ri = i_pool.tile([P, n_r // 16], i16, tag="ri")
            nc.sync.dma_start(
                ri[:], ridx_d.ap()[:, r_off:r_off + n_r // 16]
            )
            g_r = gr_pool.tile([P, su * ZW], f32, tag="gr")
            nc.gpsimd.dma_gather(
                out_ap=g_r[:].rearrange("p (u d) -> p u d", d=ZW),
                in_ap=z_r_d.ap(),
                idxs_ap=ri[:],
                num_idxs=n_r,
                num_idxs_reg=n_r,
                elem_size=ZW,
                single_packet=False,
                queue_num=qctr.pop() % 4,
            )
            return g_r

        front = [emit_row_gather(ci) for ci in range(min(NFRONT, len(layout.chunks)))]
        for ci, (kk, su, u0, r_off, c_off, o_off) in enumerate(layout.chunks):
            n_r = P * su
            n_c = P * su * kk
            X = su * kk                      # out slot-columns this chunk
            g_r = front[ci] if ci < NFRONT else emit_row_gather(ci)
            ci_t = i_pool.tile([P, n_c // 16], i16, tag="ci")
            nc.sync.dma_start(
                ci_t[:], cidx_d.ap()[:, c_off:c_off + n_c // 16]
            )
            g_c = gc_pool.tile([P, X * ZW], f32, tag="gc")
            gcv = g_c[:].rearrange("p (u d) -> p u d", d=ZW)
            nsplit = 3 if X >= 6 else (2 if X >= 2 else 1)
            bounds = [(X * j // nsplit) * P for j in range(nsplit)] + [n_c]
            for piece in range(nsplit):
                lo, hi = bounds[piece], bounds[piece + 1]
                if hi <= lo:
                    continue
                nc.gpsimd.dma_gather(
                    out_ap=gcv[:, lo // P:hi // P],
                    in_ap=z_c_d.ap(),
                    idxs_ap=ci_t[:, lo // 16:hi // 16],
                    num_idxs=hi - lo,
                    num_idxs_reg=hi - lo,
                    elem_size=ZW,
                    single_packet=False,
                    queue_num=qctr.pop() % 4,
                )
            m = m_pool.tile([P, X * DD], f32)
            nc.vector.tensor_tensor(
                out=m[:].rearrange("p (u m d) -> p u m d", u=su, m=kk),
                in0=g_r[:]
                .rearrange("p (u d) -> p u d", d=ZW)[:, :, 0:DD]
                .unsqueeze(2)
                .to_broadcast([P, su, kk, DD]),
                in1=g_c[:].rearrange("p (u m d) -> p u m d", u=su, d=ZW)[
                    :, :, :, DD:2 * DD
                ],
                op=mybir.AluOpType.add,
            )
            et = e_pool.tile([P, X * DD], f32)
            nc.scalar.activation(et[:], m[:], mybir.ActivationFunctionType.Exp)
            e3 = et[:].rearrange("p (t d) -> p t d", d=D)  # [128, X*5, 5]
            s = s_pool.tile([P, X * D], f32, tag="s")
            nc.vector.reduce_sum(s[:], e3, axis=mybir.AxisListType.X)
            r = s_pool.tile([P, X * D], f32, tag="r")
            nc.vector.reciprocal(r[:], s[:])
            o = o_pool.tile([P, X * DD], f32)
            nc.vector.tensor_tensor(
                out=o[:].rearrange("p (t d) -> p t d", d=D),
                in0=e3,
                in1=r[:].unsqueeze(2).to_broadcast([P, X * D, D]),
                op=mybir.AluOpType.mult,
            )
            nc.vector.tensor_tensor(
                out=o[:].rearrange("p (t d) -> p t d", d=DD),
                in0=eye_tile[:].unsqueeze(1).to_broadcast([P, X, DD]),
                in1=o[:].rearrange("p (t d) -> p t d", d=DD),
                op=mybir.AluOpType.subtract,
            )
            nc.sync.dma_start(
                oview[:, o_off * DD:(o_off + X) * DD], o[:]
            )

    nc.compile()
    return nc


def _wrap_idx(a):
    """Gather-order idx list [n] -> [128, n//16] int16 tile (16-wrapped,
    replicated to 128 partitions)."""
    n = len(a)
    assert n % 16 == 0
    t = a.reshape(n // 16, 16).T.astype(np.int16)   # [16, n//16]
    return np.ascontiguousarray(np.tile(t, (8, 1)))


def _host_prep(x, W, edge_index):
    """Shard edges by (row_half, col_half) class across cores; group by row
    node into multiplicity classes; build per-core inputs. Returns
    (layout, in_maps, slot_maps) where slot_maps[c] = (edge_ids, out_rows)."""
    x = np.asarray(x, dtype=np.float32)
    W = np.asarray(W, dtype=np.float32)
    ei = np.asarray(edge_index)
    row = ei[0].astype(np.int64)
    col = ei[1].astype(np.int64)

    import ml_dtypes

    bf16 = ml_dtypes.bfloat16
    xt = np.zeros((P, 2 * HALF), dtype=np.float32)
    xt[:, :N] = x.T
    xt_half = [
        np.ascontiguousarray(xt[:, :HALF].astype(bf16)),
        np.ascontiguousarray(xt[:, HALF:].astype(bf16)),
    ]

    w = np.zeros((P, 2 * DD), dtype=np.float32)
    w[:, :DD] = W[:, :C].T
    w[:, DD:2 * DD] = W[:, C:].T
    w = w.astype(bf16)
    eye = np.ascontiguousarray(
        np.broadcast_to(np.eye(D, dtype=np.float32).reshape(1, DD), (P, DD))
    )

    cls = (row >= HALF).astype(np.int64) * 2 + (col >= HALF)
    order = np.argsort(cls, kind="stable")
    counts = np.bincount(cls, minlength=4)
    starts = np.concatenate([[0], np.cumsum(counts)])

    subs = []
    row_locals = []
    for core in range(NCORES):
        k = core // 2
        sub = order[starts[k]:starts[k + 1]][core % 2::2]
        subs.append(sub)
        row_locals.append((row[sub] - (k >> 1) * HALF).astype(np.int64))

    layout, per_core_groups = _build_layout(row_locals)

    in_maps = []
    slot_maps = []
    for core in range(NCORES):
        k = core // 2
        half_r, half_c = k >> 1, k & 1
        sub = subs[core]
        col_local = (col[sub] - half_c * HALF).astype(np.int64)
        groups = per_core_groups[core]

        # per class: padded group arrays (row id + kk edge positions);
        # pads use row id -1 (trailing in each list -> SWDGE skips them)
        cls_rows = {}
        cls_edges = {}
        for kk, q in layout.quotas.items():
            g = groups.get(kk, [])
            rows_a = np.zeros(q, dtype=np.int64)
            edges_a = np.full((q, kk), -1, dtype=np.int64)
            for i, (r, pos) in enumerate(g):
                rows_a[i] = r
                edges_a[i] = pos
            cls_rows[kk] = rows_a
            cls_edges[kk] = edges_a

        # build per chunk
        r_flat = []
        c_flat = []
        eids = []
        orows = []
        t_cols = layout.t_cols
        for (kk, su, u0, r_off, c_off, o_off) in layout.chunks:
            g0 = u0 * P
            rows_a = cls_rows[kk][g0:g0 + su * P]          # [su*128]
            edges_a = cls_edges[kk][g0:g0 + su * P]        # [su*128, kk]
            r_flat.append(rows_a.astype(np.int16))          # pos = u*128+p
            # col idx order: u-major, pos = (u*kk + m)*128 + p
            e_um = edges_a.reshape(su, P, kk).transpose(0, 2, 1)  # [su, kk, P]
            cvals = np.where(e_um >= 0, col_local[np.maximum(e_um, 0)], 0)
            cvals = cvals.reshape(-1)
            c_flat.append(cvals.astype(np.int16))
            # out slot rows: edge (u, m, p) -> out row p*t_cols + (o_off + u*kk + m)
            uu, mm, pp = np.meshgrid(
                np.arange(su), np.arange(kk), np.arange(P), indexing="ij"
            )
            valid = e_um >= 0
            rws = pp * t_cols + (o_off + uu * kk + mm)
            eids.append(sub[e_um[valid]])
            orows.append(rws[valid])
        ridx = _wrap_idx(np.concatenate(r_flat))
        cidx = _wrap_idx(np.concatenate(c_flat))
        in_maps.append(
            {
                "xt_r": xt_half[half_r],
                "xt_c": xt_half[half_c],
                "w": w,
                "eye": eye,
                "ridx": ridx,
                "cidx": cidx,
            }
        )
        slot_maps.append(
            (np.concatenate(eids), np.concatenate(orows))
        )
    return layout, in_maps, slot_maps


LAST_EXEC_NS = None


def kernel(x, W, edge_index):
    global LAST_EXEC_NS
    from concourse.bass_utils import run_bass_kernel_spmd

    layout, in_maps, slot_maps = _host_prep(x, W, edge_index)
    nc = _build_nc(layout)
    trace = os.environ.get("KERNEL_TRACE", "0") == "1"
    br = run_bass_kernel_spmd(
        nc,
        in_maps,
        core_ids=list(range(NCORES)),
        trace=trace,
    )
    LAST_EXEC_NS = br.exec_time_ns

    out = np.empty((E, DD), dtype=np.float32)
    for core in range(NCORES):
        res = br.results[core]["out"]        # [P*t_cols, 25]
        eids, orows = slot_maps[core]
        out[eids] = res[orows]
    return out.reshape(E, D, D).astype(np.float32)
